# revision 17
# baseline (speedup 1.0000x reference)
"""Attention-pooling kernel (AttLayer) for Trainium2, 8 NeuronCores.

Math (per batch b):
    uit  = tanh(x @ W + b)          # [T, A]
    ait  = exp(uit @ u)             # [T]
    out  = (sum_t ait[t] * x[t,:]) / (sum_t ait[t] + EPS)   # [D]

Single pass over x: accumulate numerator and denominator together.

Device data layout (per core, pure data-parallel over batch; host casts to
bf16 and pre-transposes so x streams with d on partitions at full DMA rate):
    xt   [2, 128, BL*T] bf16  -- host-pre-transposed x: (d_chunk, d_in_chunk, b*t)
    w    [2, 128, A]    bf16  -- W split into two 128-row d-chunks
    bb   [A, 1]         f32   -- bias, per-partition for ACT
    urep [A, 128]       bf16  -- u tiled to 128 columns (logit-broadcast trick)
Outputs:
    num  [128, 2*BL]    f32   -- numerator, col = b*2 + c
    den  [1, NQ*BL]     f32   -- per-quarter exp-sum partials, col = b*NQ + qi

Pipeline per (b, t-quarter), software-pipelined by one quarter:
    PE : psum_uitT[A, TQ]   = W0^T @ xt0 + W1^T @ xt1        (contract d)
    ACT: uitT_sb            = tanh(psum_uitT + bb)           (bias per-partition)
    PE : psum_logit[128,TQ] = urep^T @ uitT_sb               (contract a; all
                              128 output rows identical = partition-broadcast)
    ACT: e_sb               = exp(psum_logit), accum_out -> den partial
    DVE: custom TENSOR_TENSOR_REDUCE(xt_c * e_sb) -> num[:, b*2+c], seeded
         with the previous quarter's accumulator (fused multiply+reduce)
Final division num/(den+EPS) happens on host during unsharding.
"""

import sys
import types

sys.path.insert(0, "/opt/trn_rl_repo")

# bass_utils' trace path imports antenv.axon_hooks, which not every image
# ships; register a no-op fallback so trace=True degrades instead of crashing.
try:
    import antenv.axon_hooks  # noqa: F401
except ImportError:
    try:
        import antenv

        _hooks = types.ModuleType("antenv.axon_hooks")
        _hooks._HOOK = None

        def _set_hook(hook):
            _hooks._HOOK = hook

        def _get_hook():
            return _hooks._HOOK

        _hooks.set_axon_ntff_profile_hook = _set_hook
        _hooks.get_axon_ntff_profile_hook = _get_hook
        sys.modules["antenv.axon_hooks"] = _hooks
        antenv.axon_hooks = _hooks
    except ImportError:
        pass

import numpy as np
import ml_dtypes

import concourse.bacc as bacc
import concourse.tile as tile
from concourse import mybir
from concourse import bass_utils
from concourse.dve_ops import TENSOR_TENSOR_REDUCE

B, T, D, A = 64, 4096, 256, 50
NCORES = 8
BL = B // NCORES  # batches per core
EPS = 1e-7
P = 128
NCH = D // P  # 2 d-chunks


def build_attpool(nc, aps, BL, T):
    """Emit the tile program. aps: dict name->AP for dram tensors."""
    TH = T // 2   # half (TTR granularity)
    TQ = T // 4   # quarter (ACT/psum granularity)
    NQ = 4
    xt, w, bb, urep = aps["xt"], aps["w"], aps["bb"], aps["urep"]
    num, den = aps["num"], aps["den"]
    f32 = mybir.dt.float32
    bf16 = mybir.dt.bfloat16

    with tile.TileContext(nc) as tc:
        with (
            tc.tile_pool(name="singles", bufs=1) as singles,
            tc.tile_pool(name="x0", bufs=8) as x0_pool,
            tc.tile_pool(name="x1", bufs=8) as x1_pool,
            tc.tile_pool(name="uitT", bufs=3) as uitT_pool,
            tc.tile_pool(name="e", bufs=4) as e_pool,
            tc.tile_pool(name="scratch", bufs=3) as scratch_pool,
            tc.tile_pool(name="ps_uitT", bufs=2, space="PSUM") as ps_uitT_pool,
            tc.tile_pool(name="ps_logit", bufs=2, space="PSUM") as ps_logit_pool,
        ):
            # constants
            w_sb = [
                singles.tile([P, A], bf16, tag=f"w{c}", name=f"w_sb{c}")
                for c in range(NCH)
            ]
            for c in range(NCH):
                nc.sync.dma_start(out=w_sb[c][:, :], in_=w[c, :, :])
            bb_sb = singles.tile([A, 1], f32)
            urep_sb = singles.tile([A, P], bf16)
            num_sb = singles.tile([P, NCH * BL], f32)
            den_sb = singles.tile([P, NQ * BL], f32)

            # flatten all quarters; software-pipeline by one stage so PE's
            # step1(i+1) is emitted before step2(i): keeps PE/ACT streaming
            # instead of ping-ponging on the s1->tanh->s2->exp chain.
            quarters = [
                (b, h, q) for b in range(BL) for h in range(2) for q in range(2)
            ]

            xt_tiles = {}   # (b, h) -> [tile_c0, tile_c1], each [P, TH]
            e_tiles = {}    # (b, h) -> e tile [P, TH]

            def load_half(b, h):
                xt_t = [None, None]
                for c, pool in ((0, x0_pool), (1, x1_pool)):
                    xt_t[c] = pool.tile(
                        [P, TH], bf16, tag=f"xt{c}", name=f"xt_t{c}"
                    )
                    nc.sync.dma_start(
                        out=xt_t[c][:, :],
                        in_=xt[c, :, b * T + h * TH : b * T + (h + 1) * TH],
                    )
                xt_tiles[(b, h)] = xt_t

            def stage1(b, h, q):
                if (b, h) not in xt_tiles:
                    load_half(b, h)
                xt_t = xt_tiles[(b, h)]
                off = q * TQ
                ps_uitT = ps_uitT_pool.tile([A, TQ], f32, tag="psu")
                # W0 for both 512-blocks, then W1: adjacent same-weight MMs
                for c in range(NCH):
                    for s in range(0, TQ, 512):
                        sw = min(512, TQ - s)
                        nc.tensor.matmul(
                            ps_uitT[:, s : s + sw],
                            lhsT=w_sb[c][:, :],
                            rhs=xt_t[c][:, off + s : off + s + sw],
                            start=(c == 0),
                            stop=(c == NCH - 1),
                        )
                return ps_uitT

            def stage2(state):
                (b, h, q), ps_uitT = state
                qi = h * 2 + q
                uitT_sb = uitT_pool.tile([A, TQ], bf16, tag="uitT")
                nc.scalar.activation(
                    uitT_sb[:, :], ps_uitT[:, :],
                    mybir.ActivationFunctionType.Tanh,
                    bias=bb_sb[:, :],
                )
                ps_logit = ps_logit_pool.tile([P, TQ], f32, tag="psl")
                for s in range(0, TQ, 512):
                    sw = min(512, TQ - s)
                    nc.tensor.matmul(
                        ps_logit[:, s : s + sw],
                        lhsT=urep_sb[:, :],
                        rhs=uitT_sb[:, s : s + sw],
                        start=True,
                        stop=True,
                    )
                if (b, h) not in e_tiles:
                    e_tiles[(b, h)] = e_pool.tile([P, TH], f32, tag="e", name="e_sb")
                e_sb = e_tiles[(b, h)]
                nc.scalar.activation(
                    e_sb[:, q * TQ : (q + 1) * TQ], ps_logit[:, :],
                    mybir.ActivationFunctionType.Exp,
                    accum_out=den_sb[:, b * NQ + qi : b * NQ + qi + 1],
                )
                # per-quarter fused multiply+reduce, chained across quarters
                xt_t = xt_tiles[(b, h)]
                for c in range(NCH):
                    col = b * NCH + c
                    scr = scratch_pool.tile([P, TQ], bf16, tag="scr")
                    nc.vector._custom_dve(
                        TENSOR_TENSOR_REDUCE,
                        out=scr[:, :],
                        in0=xt_t[c][:, q * TQ : (q + 1) * TQ],
                        in1=e_sb[:, q * TQ : (q + 1) * TQ],
                        s0=0.0 if qi == 0 else num_sb[:, col : col + 1],
                        s1=1.0,
                        accum_out=num_sb[:, col : col + 1],
                    )
                if q == 1:
                    xt_tiles.pop((b, h))
                    del e_tiles[(b, h)]
                if qi == 3 and b == BL - 2:
                    # drain all finished batches' numerators early so only
                    # the last batch's 2 columns remain for the tail DMA
                    nc.sync.dma_start(
                        out=num[:, : (BL - 1) * NCH],
                        in_=num_sb[:, : (BL - 1) * NCH],
                    )

            pend = None
            first = True
            for qd in quarters:
                ps = stage1(*qd)
                if first:
                    # bias/urep consts load after the first x tiles so the
                    # critical first matmul's data is in front of the queue
                    nc.sync.dma_start(out=bb_sb[:, :], in_=bb[:, :])
                    nc.sync.dma_start(out=urep_sb[:, :], in_=urep[:, :])
                    first = False
                if pend is not None:
                    stage2(pend)
                pend = (qd, ps)
            stage2(pend)
            nc.sync.dma_start(
                out=num[:, (BL - 1) * NCH :], in_=num_sb[:, (BL - 1) * NCH :]
            )
            nc.sync.dma_start(out=den[:, :], in_=den_sb[0:1, :])
    return nc


def _declare(nc, BL, T):
    f32 = mybir.dt.float32
    bf16 = mybir.dt.bfloat16
    NQ = 4
    aps = {
        "xt": nc.dram_tensor("xt", (NCH, P, BL * T), bf16, kind="ExternalInput").ap(),
        "w": nc.dram_tensor("w", (NCH, P, A), bf16, kind="ExternalInput").ap(),
        "bb": nc.dram_tensor("bb", (A, 1), f32, kind="ExternalInput").ap(),
        "urep": nc.dram_tensor("urep", (A, P), bf16, kind="ExternalInput").ap(),
        "num": nc.dram_tensor("num", (P, NCH * BL), f32, kind="ExternalOutput").ap(),
        "den": nc.dram_tensor("den", (1, NQ * BL), f32, kind="ExternalOutput").ap(),
    }
    return aps


_CACHE = {}


def _get_nc():
    key = "nc"
    if key not in _CACHE:
        nc = bacc.Bacc(
            "TRN2", target_bir_lowering=False, debug=False,
            enable_asserts=False, num_devices=NCORES,
        )
        aps = _declare(nc, BL, T)
        build_attpool(nc, aps, BL, T)
        nc.compile()
        _CACHE[key] = nc
    return _CACHE[key]


def _host_prep(x, W, b, u):
    """Build per-core input maps from full inputs."""
    x = np.asarray(x, dtype=np.float32)
    W = np.asarray(W, dtype=np.float32)
    b = np.asarray(b, dtype=np.float32)
    u = np.asarray(u, dtype=np.float32)
    wc = np.ascontiguousarray(W.reshape(NCH, P, A)).astype(ml_dtypes.bfloat16)
    bb = np.ascontiguousarray(b.reshape(A, 1))
    urep = np.ascontiguousarray(np.tile(u.reshape(A, 1), (1, P))).astype(ml_dtypes.bfloat16)
    in_maps = []
    for core in range(NCORES):
        xc = x[core * BL : (core + 1) * BL]  # [BL, T, D]
        # -> [NCH, P, BL*T]: xt[c, dp, b*T+t] = x[b, t, c*128+dp]
        xt = np.ascontiguousarray(
            xc.reshape(BL, T, NCH, P).transpose(2, 3, 0, 1).reshape(NCH, P, BL * T)
        ).astype(ml_dtypes.bfloat16)
        in_maps.append({"xt": xt, "w": wc, "bb": bb, "urep": urep})
    return in_maps


def _unshard(results):
    out = np.empty((B, D), dtype=np.float32)
    NQ = 4
    for core in range(NCORES):
        num = results[core]["num"]          # [128, 2*BL]
        den = results[core]["den"]          # [1, NQ*BL]
        den_b = den.reshape(BL, NQ).sum(axis=1)  # [BL]
        for bl in range(BL):
            vec = np.concatenate(
                [num[:, bl * NCH + c] for c in range(NCH)]
            )  # [D]
            out[core * BL + bl] = vec / (den_b[bl] + EPS)
    return out


def kernel(x, W, b, u, _trace=False):
    nc = _get_nc()
    in_maps = _host_prep(x, W, b, u)
    res = bass_utils.run_bass_kernel_spmd(
        nc, in_maps, core_ids=list(range(NCORES)), trace=_trace,
    )
    out = _unshard(res.results)
    if _trace:
        kernel.last_result = res
    return out
